# revision 1
# baseline (speedup 1.0000x reference)
"""Trainium2 Bass kernel for paged causal self-attention (GQA + YaRN rope).

Sharding: tensor-parallel over heads. Core c (of 8) owns kv-head c and
q-heads 2c, 2c+1 for both batches. Each core computes a partial output
y_c = attn_c @ Wo_c.T over its 256 channels; the host sums the 8 partials.

The reference's scatter of new K/V into the pools is dead code w.r.t. the
returned output (slot_map is a permutation, so gathered past slots are
disjoint from the scattered new slots); new K/V are consumed directly from
SBUF and only the past 1024 slots per batch are gathered via indirect DMA.

Matmuls run as float32r (full PE rate at free dim >= 256). The BIR verifier
requires every fp32r matmul operand to be produced by a compute op that
rounds to fp32r, so DMA-loaded operands pass through a rounding copy and
intermediate tiles are declared float32r at their producing op.
"""

import sys

sys.path.insert(0, "/opt/trn_rl_repo")

import numpy as np

import concourse.bacc as bacc
import concourse.bass as bass
import concourse.tile as tile
from concourse import mybir
from concourse.bass import IndirectOffsetOnAxis
from concourse.bass_utils import run_bass_kernel_spmd

F32 = mybir.dt.float32
F32R = mybir.dt.float32r
I32 = mybir.dt.int32
EXP = mybir.ActivationFunctionType.Exp

B, T, PAST = 2, 1024, 1024
H, HKV, D = 16, 8, 128
G = H // HKV            # q heads per kv head
C = H * D               # 2048
TOTAL = PAST + T        # 2048
NB = B * T              # 2048 flattened tokens
NCORES = 8
P = 128
TB = 512                # token block for projections
NEG = -1.0e30


def _f(ap):
    return ap.bitcast(F32)


def _emit(tc, io):
    nc = tc.nc
    (xT, wq, wk, wv, wo, kp, vp, gidx, cosq, sinq, cosk, sink,
     cmask, rperm, ident, ones, y) = io

    with (
        tc.tile_pool(name="const", bufs=1) as cp,
        tc.tile_pool(name="persist", bufs=1) as pp,
        tc.tile_pool(name="gather", bufs=1) as gp,
        tc.tile_pool(name="ysb", bufs=3) as yp,
    ):
        # ---- constants ----
        gidx_t = cp.tile([P, 2 * 8], I32)
        nc.sync.dma_start(gidx_t[:], gidx[:])
        cmask_t = cp.tile([P, 4, TB], F32)
        nc.sync.dma_start(cmask_t[:], cmask[:])
        ident_t = cp.tile([P, P], F32)
        nc.sync.dma_start(ident_t[:], ident[:])
        rperm_s = cp.tile([P, P], F32)
        nc.sync.dma_start(rperm_s[:], rperm[:])
        rperm_t = cp.tile([P, P], F32R)
        nc.vector.tensor_copy(rperm_t[:], rperm_s[:])
        ones_s = cp.tile([P, P], F32)
        nc.sync.dma_start(ones_s[:], ones[:])
        ones_t = cp.tile([P, P], F32R)
        nc.vector.tensor_copy(ones_t[:], ones_s[:])

        vgr = [None, None]

        # ---- persistent activations (float32r: producers are compute ops) ----
        qT0 = pp.tile([P, NB], F32R)      # q head 2c,   [d, token]
        qT1 = pp.tile([P, NB], F32R)      # q head 2c+1
        kT_new = pp.tile([P, NB], F32R)   # new keys,    [d, token]
        v_nat = pp.tile([P, B, 8, P], F32R)    # new values, [t%128, b, chunk, d]
        kT_past = pp.tile([P, B, 8, P], F32R)  # past keys,  [d, b, chunk, s%128]
        wo_t = pp.tile([P, G, C], F32R)   # rounded Wo slice

        # ================= phase 1: projections + rope =================
        with (
            tc.tile_pool(name="tabs", bufs=1) as tbp,
            tc.tile_pool(name="kgvg", bufs=1) as kvp,
            tc.tile_pool(name="wts", bufs=1) as wp,
            tc.tile_pool(name="xt", bufs=3) as xp,
            tc.tile_pool(name="rope", bufs=2) as rp,
            tc.tile_pool(name="pproj", bufs=1, space="PSUM") as pjp,
            tc.tile_pool(name="prope", bufs=2, space="PSUM") as rpp,
            tc.tile_pool(name="ptr", bufs=2, space="PSUM") as trp,
        ):
            # past K/V gather (emit early; DMA overlaps weight loads)
            kg = [None, None]
            for b in range(B):
                kg[b] = kvp.tile([P, 8, P], F32, name=f"kg{b}", tag=f"kg{b}")
                vg = kvp.tile([P, 8, P], F32, name=f"vg{b}", tag=f"vg{b}")
                for j in range(8):
                    # [P,1]-index gathers: the multi-column offset-AP form
                    # miscomputes on hardware
                    nc.gpsimd.indirect_dma_start(
                        out=kg[b][:, j, :],
                        out_offset=None,
                        in_=kp[:, :],
                        in_offset=IndirectOffsetOnAxis(
                            ap=gidx_t[:, 8 * b + j:8 * b + j + 1], axis=0),
                    )
                    nc.gpsimd.indirect_dma_start(
                        out=vg[:, j, :],
                        out_offset=None,
                        in_=vp[:, :],
                        in_offset=IndirectOffsetOnAxis(
                            ap=gidx_t[:, 8 * b + j:8 * b + j + 1], axis=0),
                    )
                vgr[b] = gp.tile([P, 8, P], F32R, name=f"vgr{b}", tag=f"vgr{b}")
                nc.vector.tensor_copy(vgr[b][:], vg[:])

            # rope tables (phase-1 only)
            cosq_t = tbp.tile([P, T], F32)
            nc.sync.dma_start(cosq_t[:], cosq[:])
            sinq_t = tbp.tile([P, T], F32)
            nc.sync.dma_start(sinq_t[:], sinq[:])
            cosk_t = tbp.tile([P, T], F32)
            nc.sync.dma_start(cosk_t[:], cosk[:])
            sink_t = tbp.tile([P, T], F32)
            nc.sync.dma_start(sink_t[:], sink[:])

            # weights: stage in [P,4,P] pieces through the xts slot rotation
            # (24 pieces = 3 full cycles of 8 slots, keeping the HWDGE
            # round-robin aligned so slot-reuse WAW deps stay same-proc)
            wq_t = wp.tile([P, 16, 2 * P], F32R)
            wk_t = wp.tile([P, 16, P], F32R)
            wv_t = wp.tile([P, 16, P], F32R)
            wqr = wq.rearrange("(kc p) m -> p kc m", p=P)
            wkr = wk.rearrange("(kc p) m -> p kc m", p=P)
            wvr = wv.rearrange("(kc p) m -> p kc m", p=P)
            wor = wo.rearrange("(g p) (q m) -> p g q m", p=P, m=P)  # [P,2,16,128]
            pieces = []
            for hh in range(2):
                for q4 in range(4):
                    pieces.append((wqr[:, 4 * q4:4 * q4 + 4, hh * P:(hh + 1) * P],
                                   wq_t[:, 4 * q4:4 * q4 + 4, hh * P:(hh + 1) * P]))
            for q4 in range(4):
                pieces.append((wkr[:, 4 * q4:4 * q4 + 4, :],
                               wk_t[:, 4 * q4:4 * q4 + 4, :]))
            for q4 in range(4):
                pieces.append((wvr[:, 4 * q4:4 * q4 + 4, :],
                               wv_t[:, 4 * q4:4 * q4 + 4, :]))
            wot4 = wo_t.rearrange("p g (q m) -> p g q m", m=P)   # [P,2,16,128]
            for g in range(G):
                for q4 in range(4):
                    pieces.append((wor[:, g, 4 * q4:4 * q4 + 4, :],
                                   wot4[:, g, 4 * q4:4 * q4 + 4, :]))
            for src_ap, dst_ap in pieces:
                w_s = xp.tile([P, 4, P], F32, name="w_s", tag="xts", bufs=8)
                nc.sync.dma_start(w_s[:], src_ap)
                nc.vector.tensor_copy(dst_ap, w_s[:])

            for tb in range(NB // TB):           # 4 token blocks of 512
                n0 = tb * TB
                b = tb // 2
                tpos = (tb % 2) * TB             # position-in-batch of block start

                q0p = pjp.tile([P, TB], F32, name="q0p", tag="q0")
                q1p = pjp.tile([P, TB], F32, name="q1p", tag="q1")
                kkp = pjp.tile([P, TB], F32, name="kkp", tag="kk")
                vvp = pjp.tile([P, TB], F32, name="vvp", tag="vv")
                for kc in range(16):
                    xt_s = xp.tile([P, TB], F32, name="xt_s", tag="xts", bufs=8)
                    nc.sync.dma_start(xt_s[:], xT[kc * P:(kc + 1) * P, n0:n0 + TB])
                    xt = xp.tile([P, TB], F32R, name="xt", tag="xt")
                    nc.vector.tensor_copy(xt[:], xt_s[:])
                    st = (kc == 0)
                    sp = (kc == 15)
                    nc.tensor.matmul(q0p[:], wq_t[:, kc, 0:P], xt[:], start=st, stop=sp)
                    nc.tensor.matmul(q1p[:], wq_t[:, kc, P:2 * P], xt[:], start=st, stop=sp)
                    nc.tensor.matmul(kkp[:], wk_t[:, kc, :], xt[:], start=st, stop=sp)
                    nc.tensor.matmul(vvp[:], wv_t[:, kc, :], xt[:], start=st, stop=sp)

                # rope for q0, q1, k
                for src, dst, ct, stt in (
                    (q0p, qT0, cosq_t, sinq_t),
                    (q1p, qT1, cosq_t, sinq_t),
                    (kkp, kT_new, cosk_t, sink_t),
                ):
                    raw = rp.tile([P, TB], F32R, name="raw", tag="raw")
                    nc.scalar.copy(raw[:], src[:])
                    rot = rpp.tile([P, TB], F32, name="rot", tag="rot")
                    nc.tensor.matmul(rot[:], rperm_t[:], raw[:], start=True, stop=True)
                    dslice = dst[:, n0:n0 + TB]
                    nc.vector.tensor_mul(dslice, _f(raw[:]), ct[:, tpos:tpos + TB])
                    tmp = rp.tile([P, TB], F32, name="tmp", tag="tmp")
                    nc.vector.tensor_mul(tmp[:], rot[:], stt[:, tpos:tpos + TB])
                    nc.vector.tensor_add(dslice, _f(dslice), tmp[:])

                # v: no rope; transpose [d, t] -> [t, d] in 128-chunks
                vraw = rp.tile([P, TB], F32, name="vraw", tag="vraw")
                nc.scalar.copy(vraw[:], vvp[:])
                for j4 in range(TB // P):
                    vt = trp.tile([P, P], F32, name="vt", tag="tr")
                    nc.tensor.transpose(vt[:], vraw[:, j4 * P:(j4 + 1) * P],
                                        ident_t[:])
                    nc.vector.tensor_copy(v_nat[:, b, (tb % 2) * 4 + j4, :], vt[:])

            # past K transpose: [s, d] -> [d, s]
            for b in range(B):
                for j in range(8):
                    kt = trp.tile([P, P], F32, name="kt", tag="tr")
                    nc.tensor.transpose(kt[:], kg[b][:, j, :], ident_t[:])
                    nc.vector.tensor_copy(kT_past[:, b, j, :], kt[:])

        # ================= phase 3+4: attention + output proj =================
        with (
            tc.tile_pool(name="attp", bufs=1) as ap_,
            tc.tile_pool(name="exps", bufs=1) as ep,
            tc.tile_pool(name="sums", bufs=2) as sp_,
            tc.tile_pool(name="pscore", bufs=2, space="PSUM") as scp,
            tc.tile_pool(name="pav", bufs=2, space="PSUM") as avp,
            tc.tile_pool(name="psum1", bufs=1, space="PSUM") as s1p,
            tc.tile_pool(name="pbc", bufs=1, space="PSUM") as bcp,
            tc.tile_pool(name="py", bufs=2, space="PSUM") as pyp,
        ):
            att0 = ap_.tile([P, NB], F32R)    # attention out head 2c, [d, token]
            att1 = ap_.tile([P, NB], F32R)

            for b in range(B):
                for tbq in range(2):             # query block of 512 within batch
                    t0 = b * T + tbq * TB        # global token offset
                    for g, (qT, att) in enumerate(((qT0, att0), (qT1, att1))):
                        q_ap = qT[:, t0:t0 + TB]
                        njnew = 4 * tbq + 4
                        nch = 8 + njnew
                        expS = ep.tile([P, 16, TB], F32R, name="expS", tag="expS")
                        sumP = sp_.tile([P, TB], F32R, name="sumP", tag="sumP")
                        av = avp.tile([P, TB], F32, name="av", tag="av")

                        chunks = [(kT_past[:, b, j, :], vgr[b][:, j, :], None)
                                  for j in range(8)]
                        for j in range(njnew):
                            koff = b * T + j * P
                            ri = j - 4 * tbq
                            chunks.append((kT_new[:, koff:koff + P],
                                           v_nat[:, b, j, :],
                                           ri if ri >= 0 else None))

                        for ci, (k_ap, v_ap, mri) in enumerate(chunks):
                            s_ps = scp.tile([P, TB], F32, name="s_ps", tag="s")
                            nc.tensor.matmul(s_ps[:], k_ap, q_ap,
                                             start=True, stop=True)
                            if mri is not None:
                                nc.vector.tensor_add(s_ps[:], s_ps[:],
                                                     cmask_t[:, mri, :])
                            e_ap = expS[:, ci, :]
                            nc.scalar.activation(e_ap, s_ps[:], EXP)
                            if ci == 0:
                                nc.vector.tensor_copy(sumP[:], _f(e_ap))
                            else:
                                nc.vector.tensor_add(sumP[:], _f(sumP[:]), _f(e_ap))
                            nc.tensor.matmul(av[:], v_ap, e_ap,
                                             start=(ci == 0), stop=(ci == nch - 1))

                        # softmax denominator: reduce over partitions + bcast
                        tsum = s1p.tile([1, TB], F32, name="tsum", tag="t1")
                        nc.tensor.matmul(tsum[:], ones_t[:, 0:1], sumP[:],
                                         start=True, stop=True)
                        ssb = sp_.tile([1, TB], F32, name="ssb", tag="ssb")
                        nc.scalar.copy(ssb[:], tsum[:])
                        rinv = sp_.tile([1, TB], F32, name="rinv", tag="rinv")
                        nc.vector.reciprocal(rinv[:], ssb[:])
                        rinvr = sp_.tile([1, TB], F32R, name="rinvr", tag="rinvr")
                        nc.vector.tensor_copy(rinvr[:], rinv[:])
                        rbc = bcp.tile([P, TB], F32, name="rbc", tag="rbc")
                        nc.tensor.matmul(rbc[:], ones_t[0:1, :], rinvr[:],
                                         start=True, stop=True)
                        rbs = sp_.tile([P, TB], F32, name="rbs", tag="rbs")
                        nc.scalar.copy(rbs[:], rbc[:])
                        nc.vector.tensor_mul(att[:, t0:t0 + TB], av[:], rbs[:])

                    # output projection for these 512 tokens (4 chunks of 128)
                    for tc4 in range(4):
                        tt0 = t0 + tc4 * P
                        for cb in range(4):
                            yps = pyp.tile([P, TB], F32, name="yps", tag="y")
                            nc.tensor.matmul(yps[:], att0[:, tt0:tt0 + P],
                                             wo_t[:, 0, cb * TB:(cb + 1) * TB],
                                             start=True, stop=False)
                            nc.tensor.matmul(yps[:], att1[:, tt0:tt0 + P],
                                             wo_t[:, 1, cb * TB:(cb + 1) * TB],
                                             start=False, stop=True)
                            ysb = yp.tile([P, TB], F32, name="ysbt", tag="ysbt")
                            nc.scalar.copy(ysb[:], yps[:])
                            nc.sync.dma_start(
                                y[tt0:tt0 + P, cb * TB:(cb + 1) * TB], ysb[:])


def build_nc():
    nc = bacc.Bacc("TRN2")
    xT = nc.dram_tensor("xT", [C, NB], F32, kind="ExternalInput")
    wq = nc.dram_tensor("wq", [C, G * D], F32, kind="ExternalInput")
    wk = nc.dram_tensor("wk", [C, D], F32, kind="ExternalInput")
    wv = nc.dram_tensor("wv", [C, D], F32, kind="ExternalInput")
    wo = nc.dram_tensor("wo", [G * D, C], F32, kind="ExternalInput")
    kp = nc.dram_tensor("kp", [B * TOTAL, D], F32, kind="ExternalInput")
    vp = nc.dram_tensor("vp", [B * TOTAL, D], F32, kind="ExternalInput")
    gidx = nc.dram_tensor("gidx", [P, B * 8], I32, kind="ExternalInput")
    cosq = nc.dram_tensor("cosq", [P, T], F32, kind="ExternalInput")
    sinq = nc.dram_tensor("sinq", [P, T], F32, kind="ExternalInput")
    cosk = nc.dram_tensor("cosk", [P, T], F32, kind="ExternalInput")
    sink = nc.dram_tensor("sink", [P, T], F32, kind="ExternalInput")
    cmask = nc.dram_tensor("cmask", [P, 4, TB], F32, kind="ExternalInput")
    rperm = nc.dram_tensor("rperm", [P, P], F32, kind="ExternalInput")
    ident = nc.dram_tensor("ident", [P, P], F32, kind="ExternalInput")
    ones = nc.dram_tensor("ones", [P, P], F32, kind="ExternalInput")
    y = nc.dram_tensor("y", [NB, C], F32, kind="ExternalOutput")
    io = (xT, wq, wk, wv, wo, kp, vp, gidx, cosq, sinq, cosk, sink,
          cmask, rperm, ident, ones, y)
    with nc.allow_low_precision(reason="float32r rounding for PE operands"):
        with tile.TileContext(nc) as tc:
            _emit(tc, io)
    nc.compile()
    return nc


def host_inputs(x, Wq, Wkv, Wo, K_pool, V_pool, slot_map, past_len):
    x = np.ascontiguousarray(np.asarray(x, dtype=np.float32))
    Wq = np.asarray(Wq, dtype=np.float32)
    Wkv = np.asarray(Wkv, dtype=np.float32)
    Wo = np.asarray(Wo, dtype=np.float32)
    K_pool = np.asarray(K_pool, dtype=np.float32)
    V_pool = np.asarray(V_pool, dtype=np.float32)
    slot_map = np.asarray(slot_map, dtype=np.int32)
    past = int(past_len)
    assert past == PAST, f"kernel hardcodes past_len={PAST}, got {past}"

    xT = np.ascontiguousarray(x.reshape(NB, C).T)

    # rope tables; argument arithmetic mirrors the f32 ops of the reference
    idx = np.arange(D // 2, dtype=np.float32)
    inv = np.float32(1.0) / np.float32(10000.0) ** (idx / np.float32(D // 2))
    inv = inv.astype(np.float32)
    t = np.arange(past, past + T, dtype=np.float32)
    freqs = (t[:, None] * inv[None, :]).astype(np.float32)
    emb = np.concatenate([freqs, freqs], axis=1)
    cos = np.cos(emb).astype(np.float32)
    sin = np.sin(emb).astype(np.float32)
    qscale = np.float32(1.0) / np.sqrt(np.float32(D))
    cosqT = np.ascontiguousarray((cos * qscale).T)
    sinqT = np.ascontiguousarray((sin * qscale).T)
    coskT = np.ascontiguousarray(cos.T)
    sinkT = np.ascontiguousarray(sin.T)

    s_i = np.arange(P, dtype=np.int64)[:, None]
    t_i = np.arange(TB, dtype=np.int64)[None, :]
    cm = np.empty((P, 4, TB), np.float32)
    for ri in range(4):
        cm[:, ri, :] = np.where(s_i <= t_i - ri * P, 0.0, NEG)

    gidx = slot_map[:, :past].reshape(B, 8, P).transpose(2, 0, 1).reshape(P, B * 8)
    gidx = np.ascontiguousarray(gidx.astype(np.int32))

    rperm = np.zeros((P, P), np.float32)
    for d in range(D // 2):
        rperm[d + D // 2, d] = -1.0       # rot(q)[d] = -q[d+64] for d < 64
        rperm[d, d + D // 2] = 1.0        # rot(q)[d] = q[d-64] for d >= 64
    ident = np.eye(P, dtype=np.float32)
    ones = np.ones((P, P), np.float32)

    in_maps = []
    for c in range(NCORES):
        in_maps.append({
            "xT": xT,
            "wq": np.ascontiguousarray(Wq[G * D * c:G * D * (c + 1), :].T),
            "wk": np.ascontiguousarray(Wkv[D * c:D * (c + 1), :].T),
            "wv": np.ascontiguousarray(Wkv[HKV * D + D * c:HKV * D + D * (c + 1), :].T),
            "wo": np.ascontiguousarray(Wo[:, G * D * c:G * D * (c + 1)].T),
            "kp": np.ascontiguousarray(K_pool[:, c, :]),
            "vp": np.ascontiguousarray(V_pool[:, c, :]),
            "gidx": gidx,
            "cosq": cosqT, "sinq": sinqT, "cosk": coskT, "sink": sinkT,
            "cmask": cm, "rperm": rperm, "ident": ident, "ones": ones,
        })
    return in_maps


_NC_CACHE = None


def kernel(**inputs):
    global _NC_CACHE
    in_maps = host_inputs(**inputs)
    if _NC_CACHE is None:
        _NC_CACHE = build_nc()
    res = run_bass_kernel_spmd(_NC_CACHE, in_maps, core_ids=list(range(NCORES)))
    y = res.results[0]["y"].astype(np.float32)
    for c in range(1, NCORES):
        y = y + res.results[c]["y"]
    return y.reshape(B, T, C)



# revision 3
# speedup vs baseline: 1.0045x; 1.0045x over previous
"""Trainium2 Bass kernel for paged causal self-attention (GQA + YaRN rope).

v3 over v2:
- Both q-heads processed per matmul: qT/att hold [d, g, token]; scores, the
  exp-sum ones-reduction, and att@V run with 1024-wide moving operands.
- Causally-masked diagonal chunks compute only the live token suffix per
  head, with one shared [128,128] lower-triangle multiplicative mask.
- Output projection for block i is emitted after attention of block i+1, so
  the PE never stalls on the softmax-denominator chain.
- reciprocal_approx_fast (18-bit) replaces the 8-pass iterative reciprocal.
- av is evacuated to SBUF by ScalarE so its PSUM slot recycles quickly;
  score/broadcast/out-proj PSUM tiles share one rotating 2-bank pool.
- x/weight DMAs are split and emitted critical-first so the PE starts ~4us in.

Sharding: tensor-parallel over heads. Core c (of 8) owns kv-head c and
q-heads 2c, 2c+1 for both batches; host sums the 8 bf16 partial y's in fp32.

The reference's scatter of new K/V into the pools is dead code w.r.t. the
returned output (slot_map is a permutation, so gathered past slots are
disjoint from the scattered new slots); new K/V are consumed directly from
SBUF and only the past 1024 slots per batch are gathered via indirect DMA,
unordered (softmax is permutation-invariant over fully-visible keys).
"""

import sys

sys.path.insert(0, "/opt/trn_rl_repo")

import ml_dtypes
import numpy as np

import concourse.bacc as bacc
import concourse.tile as tile
from concourse import mybir
from concourse.bass import IndirectOffsetOnAxis
from concourse.bass_utils import run_bass_kernel_spmd

BF = mybir.dt.bfloat16
F32 = mybir.dt.float32
I32 = mybir.dt.int32
EXP = mybir.ActivationFunctionType.Exp

B, T, PAST = 2, 1024, 1024
H, HKV, D = 16, 8, 128
G = H // HKV            # q heads per kv head
C = H * D               # 2048
TOTAL = PAST + T        # 2048
NB = B * T              # 2048 flattened tokens
NCORES = 8
P = 128
TB = 512                # token block
NTB = NB // TB          # 4


def _emit(tc, io):
    nc = tc.nc
    (xim, wq, wk, wv, wo, kp, vp, gidx, rope, cmask, aux, y) = io

    with (
        tc.tile_pool(name="const", bufs=1) as cp,
        tc.tile_pool(name="persist", bufs=1) as pp,
    ):
        # dma_start costs ~0.6us of serial issue time on its HWDGE engine, so
        # keep the DMA count low and split issues across sync AND scalar (both
        # are HWDGE on TRN2). Criticality order: kc=0..3 matmuls need
        # xq0/wq-half0 (sync) + wk/wv (scalar).
        wq2 = []
        for h in range(2):
            wqh = cp.tile([P, 8, G * P], BF, name=f"wqh{h}", tag=f"wqh{h}")
            wq2.append(wqh)
        wk_t = cp.tile([P, 16, P], BF)
        wv_t = cp.tile([P, 16, P], BF)
        nc.scalar.dma_start(wk_t[:], wk[:])
        nc.scalar.dma_start(wv_t[:], wv[:])
        nc.scalar.dma_start(wq2[1][:], wq[:, 8:16, :])
        nc.sync.dma_start(wq2[0][:], wq[:, 0:8, :])
        gidx_t = cp.tile([P, 2 * 8], I32)
        nc.scalar.dma_start(gidx_t[:], gidx[:])
        aux_t = cp.tile([P, 3, P], BF)          # rperm | ident | ones
        nc.scalar.dma_start(aux_t[:], aux[:])
        rope_t = cp.tile([P, 4, T], BF)         # cosq*s | sinq*s | cosk | sink
        nc.scalar.dma_start(rope_t[:], rope[:])

        # ---- persistent activations ----
        # per-token-block tiles: Tile tracks dependencies per whole tile, so
        # monolithic tensors would chain early attention blocks behind the
        # last block's projection/rope writes
        qT_blk = [pp.tile([P, G, TB], BF, name=f"qT{i}", tag=f"qT{i}")
                  for i in range(NTB)]      # roped q, [d, g, token]
        kT_blk = [pp.tile([P, TB], BF, name=f"kTn{i}", tag=f"kTn{i}")
                  for i in range(NTB)]      # new keys, [d, token]
        v_blk = [pp.tile([P, 4, P], BF, name=f"vn{i}", tag=f"vn{i}")
                 for i in range(NTB)]       # new values, [t%128, chunk, d]
        kT_past_b = [pp.tile([P, 8, P], BF, name=f"kTp{b}", tag=f"kTp{b}")
                     for b in range(B)]     # past keys, [d, chunk, s%128]
        # one att tile per 512-token block: whole-tile dependency tracking
        # would otherwise chain block i's out-proj behind block i+1's
        # normalize write
        att_blk = [pp.tile([P, G, TB], BF, name=f"att{i}", tag=f"att{i}")
                   for i in range(NTB)]
        kg = [None, None]
        vg = [None, None]
        for b in range(B):
            kg[b] = pp.tile([P, 8, P], BF, name=f"kg{b}", tag=f"kg{b}")
            vg[b] = pp.tile([P, 8, P], BF, name=f"vg{b}", tag=f"vg{b}")

        # ================= phase 1: projections + rope =================
        with (
            tc.tile_pool(name="xin", bufs=1) as xp,
            tc.tile_pool(name="rope_sb", bufs=2) as rp,
            tc.tile_pool(name="pproj", bufs=1, space="PSUM") as pjp,
            tc.tile_pool(name="prot", bufs=2, space="PSUM") as rpp,
            tc.tile_pool(name="ptr", bufs=2, space="PSUM") as trp,
        ):
            for tb in range(NTB):
                n0 = tb * TB
                b = tb // 2
                tpos = (tb % 2) * TB        # position-in-batch of block start

                if tb == 0:
                    # quarter-split so the first matmuls start ~3us in
                    xq = [None] * 4
                    for q4 in range(4):
                        xq[q4] = xp.tile([P, 4, TB], BF, name="xt",
                                         tag=f"xq{q4}")
                        nc.sync.dma_start(xq[q4][:],
                                          xim[tb, :, 4 * q4:4 * q4 + 4, :])
                    x_aps = [xq[kc // 4][:, kc % 4, :] for kc in range(16)]
                else:
                    xt = xp.tile([P, 16, TB], BF, name="xt16", tag="xt16",
                                 bufs=2)
                    nc.sync.dma_start(xt[:], xim[tb])
                    x_aps = [xt[:, kc, :] for kc in range(16)]

                q0p = pjp.tile([P, TB], F32, name="q0p", tag="q0")
                q1p = pjp.tile([P, TB], F32, name="q1p", tag="q1")
                kkp = pjp.tile([P, TB], F32, name="kkp", tag="kk")
                vvp = pjp.tile([P, TB], F32, name="vvp", tag="vv")
                for kc in range(16):
                    st = (kc == 0)
                    sp = (kc == 15)
                    x_ap = x_aps[kc]
                    nc.tensor.matmul(q0p[:], wq2[kc // 8][:, kc % 8, 0:P],
                                     x_ap, start=st, stop=sp)
                    nc.tensor.matmul(q1p[:], wq2[kc // 8][:, kc % 8, P:2 * P],
                                     x_ap, start=st, stop=sp)
                    nc.tensor.matmul(kkp[:], wk_t[:, kc, :], x_ap,
                                     start=st, stop=sp)
                    nc.tensor.matmul(vvp[:], wv_t[:, kc, :], x_ap,
                                     start=st, stop=sp)

                # rope for q0, q1, k
                for src, dslice, ci, si in (
                    (q0p, qT_blk[tb][:, 0, :], 0, 1),
                    (q1p, qT_blk[tb][:, 1, :], 0, 1),
                    (kkp, kT_blk[tb][:], 2, 3),
                ):
                    raw = rp.tile([P, TB], BF, name="raw", tag="raw")
                    nc.scalar.copy(raw[:], src[:])
                    rot = rpp.tile([P, TB], F32, name="rot", tag="rot")
                    nc.tensor.matmul(rot[:], aux_t[:, 0, :], raw[:],
                                     start=True, stop=True)
                    nc.vector.tensor_mul(dslice, raw[:],
                                         rope_t[:, ci, tpos:tpos + TB])
                    tmp = rp.tile([P, TB], BF, name="tmp", tag="tmp")
                    nc.vector.tensor_mul(tmp[:], rot[:],
                                         rope_t[:, si, tpos:tpos + TB])
                    nc.vector.tensor_add(dslice, dslice, tmp[:])

                # v: no rope; transpose [d, t] -> [t, d] in 128-chunks
                vraw = rp.tile([P, TB], BF, name="vraw", tag="vraw")
                nc.scalar.copy(vraw[:], vvp[:])
                for j4 in range(TB // P):
                    vt = trp.tile([P, P], BF, name="vt", tag="tr")
                    nc.tensor.transpose(vt[:], vraw[:, j4 * P:(j4 + 1) * P],
                                        aux_t[:, 1, :])
                    nc.vector.tensor_copy(v_blk[tb][:, j4, :], vt[:])

                if tb == 0:
                    # non-critical loads + past K/V gathers; emitted after
                    # tb0 so they don't contend with the startup-critical
                    # DMAs (gpsimd starts the gathers early regardless)
                    mask_t = cp.tile([P, G, P], BF)
                    nc.scalar.dma_start(mask_t[:], cmask[:])
                    wo_t = pp.tile([P, G, C], BF)
                    nc.scalar.dma_start(wo_t[:], wo[:])
                    for b2 in range(B):
                        for j in range(8):
                            # [P,1]-index gathers: the multi-column
                            # offset-AP form miscomputes on hardware
                            nc.gpsimd.indirect_dma_start(
                                out=kg[b2][:, j, :],
                                out_offset=None,
                                in_=kp[:, :],
                                in_offset=IndirectOffsetOnAxis(
                                    ap=gidx_t[:, 8 * b2 + j:8 * b2 + j + 1],
                                    axis=0),
                            )
                            nc.gpsimd.indirect_dma_start(
                                out=vg[b2][:, j, :],
                                out_offset=None,
                                in_=vp[:, :],
                                in_offset=IndirectOffsetOnAxis(
                                    ap=gidx_t[:, 8 * b2 + j:8 * b2 + j + 1],
                                    axis=0),
                            )

                if tb % 2 == 1:
                    # past K transpose [s, d] -> [d, s] for the batch whose
                    # projections just finished, so attention block (b, 0)
                    # is unblocked as early as possible
                    b2 = tb // 2
                    for j in range(8):
                        kt = trp.tile([P, P], BF, name="kt", tag="tr")
                        nc.tensor.transpose(kt[:], kg[b2][:, j, :],
                                            aux_t[:, 1, :])
                        if j % 2 == 0:
                            nc.vector.tensor_copy(kT_past_b[b2][:, j, :], kt[:])
                        else:
                            nc.scalar.copy(kT_past_b[b2][:, j, :], kt[:])

        # ================= phase 2: attention + output proj =================
        with (
            tc.tile_pool(name="exps", bufs=4) as ep,
            tc.tile_pool(name="avsb", bufs=2) as ab,
            tc.tile_pool(name="sums", bufs=2) as sp_,
            tc.tile_pool(name="ysb", bufs=3) as yp,
            tc.tile_pool(name="pbig", bufs=2, space="PSUM") as scp,
            tc.tile_pool(name="pav", bufs=1, space="PSUM") as avp,
            tc.tile_pool(name="psum1", bufs=1, space="PSUM") as s1p,
        ):
            def emit_attention(b, tbq):
                q_ap = qT_blk[2 * b + tbq]      # [d, g, tok]
                njnew = 4 * tbq + 4
                nch = 8 + njnew
                nfull = 8 + (4 if tbq == 1 else 0)   # chunks safe to pair-sum

                # (k_ap, v_ap, tok_off or None)
                chunks = [(kT_past_b[b][:, j, :], vg[b][:, j, :], None)
                          for j in range(8)]
                for j in range(njnew):
                    tbx = 2 * b + j // 4
                    ri = j - 4 * tbq
                    chunks.append((kT_blk[tbx][:, (j % 4) * P:(j % 4 + 1) * P],
                                   v_blk[tbx][:, j % 4, :],
                                   ri * P if ri >= 0 else None))

                tsum = s1p.tile([1, G, TB], F32, name="tsum", tag="t1")
                av = avp.tile([P, G, TB], F32, name="av", tag="av")

                # matmul PSUM outputs are capped at 512 fp32 (one bank), so
                # matmuls split per head; ACT/DVE ops span both heads.
                # Scores run one chunk ahead of tsum/av so the in-order PE
                # stream never waits on the ~1.1us exp latency.
                def emit_scores(k_ap, off):
                    if off is None or off == 0:
                        s_ps = scp.tile([P, G, TB], F32, name="s_ps", tag="s")
                        for g in range(G):
                            nc.tensor.matmul(s_ps[:, g, :], k_ap, q_ap[:, g, :],
                                             start=True, stop=True)
                        e = ep.tile([P, G, TB], BF, name="e", tag="e")
                        nc.scalar.activation(e[:], s_ps[:], EXP)
                        if off == 0:
                            nc.vector.tensor_mul(e[:, :, 0:P], e[:, :, 0:P],
                                                 mask_t[:])
                    else:
                        L = TB - off
                        s_ps = scp.tile([P, G, TB], F32, name="s_ps", tag="s")
                        e = ep.tile([P, G, TB], BF, name="e", tag="e")
                        for g in range(G):
                            nc.tensor.matmul(s_ps[:, g, 0:L],
                                             k_ap, q_ap[:, g, off:TB],
                                             start=True, stop=True)
                            nc.scalar.activation(e[:, g, 0:L],
                                                 s_ps[:, g, 0:L], EXP)
                        nc.vector.tensor_mul(e[:, :, 0:P], e[:, :, 0:P],
                                             mask_t[:])
                    return e

                def emit_av(e, v_ap, off, st, sp):
                    if off is None or off == 0:
                        for g in range(G):
                            nc.tensor.matmul(av[:, g, :], v_ap, e[:, g, :],
                                             start=st, stop=sp,
                                             skip_group_check=True)
                    else:
                        L = TB - off
                        for g in range(G):
                            nc.tensor.matmul(av[:, g, off:TB],
                                             v_ap, e[:, g, 0:L],
                                             start=st, stop=sp,
                                             skip_group_check=True)

                def emit_tsum(e_ap, off, st, sp):
                    if off is None or off == 0:
                        for g in range(G):
                            nc.tensor.matmul(tsum[:, g, :], aux_t[:, 2, 0:1],
                                             e_ap[:, g, :], start=st, stop=sp,
                                             skip_group_check=True)
                    else:
                        L = TB - off
                        for g in range(G):
                            nc.tensor.matmul(tsum[:, g, off:TB],
                                             aux_t[:, 2, 0:1], e_ap[:, g, 0:L],
                                             start=st, stop=sp,
                                             skip_group_check=True)

                # full-chunk pairs are pre-summed on DVE so the ones-reduction
                # runs half as many PE matmuls; all deferred emissions keep
                # the PE a chunk ahead of the exp/pair-add latency
                ntsum = nfull // 2 + (nch - nfull)
                tsum_done = 0
                tsum_q = []     # (e_ap, off)
                pend_av = None  # (e, v_ap, off, st)
                prev_e = None
                for ci, (k_ap, v_ap, off) in enumerate(chunks):
                    e = emit_scores(k_ap, off)
                    if tsum_q:
                        ea, eoff = tsum_q.pop(0)
                        emit_tsum(ea, eoff, tsum_done == 0, False)
                        tsum_done += 1
                    if pend_av is not None:
                        emit_av(pend_av[0], pend_av[1], pend_av[2],
                                pend_av[3], False)
                    pend_av = (e, v_ap, off, ci == 0)
                    if ci < nfull:
                        if ci % 2 == 1:
                            epair = ab.tile([P, G, TB], BF, name="epair",
                                            tag="epair", bufs=2)
                            nc.vector.tensor_add(epair[:], prev_e[:], e[:])
                            tsum_q.append((epair, None))
                        prev_e = e
                    else:
                        tsum_q.append((e, off))
                emit_av(pend_av[0], pend_av[1], pend_av[2], pend_av[3], True)
                for ea, eoff in tsum_q:
                    tsum_done += 1
                    emit_tsum(ea, eoff, False, tsum_done == ntsum)

                # denominator reciprocal + av evacuation (frees the PSUM
                # slots); the broadcast+normalize runs one block later so
                # the in-order PE stream never waits on this chain
                rinv32 = sp_.tile([1, G, TB], F32, name="rinv32", tag="ri32")
                nc.vector.reciprocal_approx_fast(rinv32[:], tsum[:])
                rinvb = sp_.tile([1, G, TB], BF, name="rinvb", tag="rib")
                nc.vector.tensor_copy(rinvb[:], rinv32[:])
                av_sb = ab.tile([P, G, TB], BF, name="av_sb", tag="avsb")
                nc.scalar.copy(av_sb[:], av[:])
                return rinvb, av_sb

            def emit_norm(blki, rinvb, av_sb):
                rbc = scp.tile([P, G, TB], F32, name="rbc", tag="s")
                for g in range(G):
                    nc.tensor.matmul(rbc[:, g, :], aux_t[0:1, 2, :],
                                     rinvb[:, g, :], start=True, stop=True)
                rbs = sp_.tile([P, G, TB], BF, name="rbs", tag="rbs")
                nc.scalar.copy(rbs[:], rbc[:])
                nc.vector.tensor_mul(att_blk[blki][:], av_sb[:], rbs[:])

            def emit_outproj(blki):
                b, tbq = divmod(blki, 2)
                t0 = b * T + tbq * TB
                att_t = att_blk[blki]
                for tc4 in range(4):
                    tt0 = t0 + tc4 * P
                    ysb = yp.tile([P, C], BF, name="ysbt", tag="ysbt")
                    for half in range(2):
                        yps = scp.tile([P, 2 * TB], F32, name="yps", tag="s")
                        for cb2 in range(2):
                            o0 = (2 * half + cb2) * TB
                            for g in range(G):
                                nc.tensor.matmul(yps[:, cb2 * TB:(cb2 + 1) * TB],
                                                 att_t[:, g, tc4 * P:tc4 * P + P],
                                                 wo_t[:, g, o0:o0 + TB],
                                                 start=(g == 0), stop=(g == 1))
                        dst = ysb[:, half * 2 * TB:(half + 1) * 2 * TB]
                        nc.vector.tensor_copy(dst, yps[:])
                    nc.sync.dma_start(y[tt0:tt0 + P, :], ysb[:])

            # two-deep software pipeline: normalize of block i runs under
            # attention of block i+1, out-proj of block i under block i+2,
            # so the PE stream never waits on the denominator chain
            blocks = [(b, tbq) for b in range(B) for tbq in range(2)]
            saved = []
            for i, blk in enumerate(blocks):
                saved.append(emit_attention(*blk))
                if i >= 1:
                    emit_norm(i - 1, *saved[i - 1])
                if i >= 2:
                    emit_outproj(i - 2)
            emit_norm(3, *saved[3])
            emit_outproj(2)
            emit_outproj(3)


def build_nc():
    nc = bacc.Bacc("TRN2")
    xim = nc.dram_tensor("xim", [NTB, P, 16, TB], BF, kind="ExternalInput")
    wq = nc.dram_tensor("wq", [P, 16, G * P], BF, kind="ExternalInput")
    wk = nc.dram_tensor("wk", [P, 16, P], BF, kind="ExternalInput")
    wv = nc.dram_tensor("wv", [P, 16, P], BF, kind="ExternalInput")
    wo = nc.dram_tensor("wo", [P, G, C], BF, kind="ExternalInput")
    kp = nc.dram_tensor("kp", [B * TOTAL, D], BF, kind="ExternalInput")
    vp = nc.dram_tensor("vp", [B * TOTAL, D], BF, kind="ExternalInput")
    gidx = nc.dram_tensor("gidx", [P, B * 8], I32, kind="ExternalInput")
    rope = nc.dram_tensor("rope", [P, 4, T], BF, kind="ExternalInput")
    cmask = nc.dram_tensor("cmask", [P, G, P], BF, kind="ExternalInput")
    aux = nc.dram_tensor("aux", [P, 3, P], BF, kind="ExternalInput")
    y = nc.dram_tensor("y", [NB, C], BF, kind="ExternalOutput")
    io = (xim, wq, wk, wv, wo, kp, vp, gidx, rope, cmask, aux, y)
    with nc.allow_low_precision(reason="bf16 dataflow; tolerance is 2e-2"):
        with tile.TileContext(nc) as tc:
            _emit(tc, io)
    nc.compile()
    return nc


def host_inputs(x, Wq, Wkv, Wo, K_pool, V_pool, slot_map, past_len):
    bf = ml_dtypes.bfloat16
    x = np.asarray(x, dtype=np.float32)
    Wq = np.asarray(Wq, dtype=np.float32)
    Wkv = np.asarray(Wkv, dtype=np.float32)
    Wo = np.asarray(Wo, dtype=np.float32)
    K_pool = np.asarray(K_pool, dtype=np.float32)
    V_pool = np.asarray(V_pool, dtype=np.float32)
    slot_map = np.asarray(slot_map, dtype=np.int32)
    past = int(past_len)
    assert past == PAST, f"kernel hardcodes past_len={PAST}, got {past}"

    # x image: [tb, p, kc, t] with p the in-channel within 128-chunk kc
    xT = x.reshape(NB, C).T                                  # [C, NB]
    xim = np.ascontiguousarray(
        xT.reshape(16, P, NTB, TB).transpose(2, 1, 0, 3)).astype(bf)

    # rope tables; argument arithmetic mirrors the f32 ops of the reference
    idx = np.arange(D // 2, dtype=np.float32)
    inv = np.float32(1.0) / np.float32(10000.0) ** (idx / np.float32(D // 2))
    inv = inv.astype(np.float32)
    t = np.arange(past, past + T, dtype=np.float32)
    freqs = (t[:, None] * inv[None, :]).astype(np.float32)
    emb = np.concatenate([freqs, freqs], axis=1)
    cos = np.cos(emb).astype(np.float32)                     # [T, D]
    sin = np.sin(emb).astype(np.float32)
    qscale = np.float32(1.0) / np.sqrt(np.float32(D))
    rope = np.ascontiguousarray(np.stack(
        [cos.T * qscale, sin.T * qscale, cos.T, sin.T], axis=1)).astype(bf)

    # one lower-triangle-inclusive 0/1 block, duplicated per q-head
    s_i = np.arange(P, dtype=np.int64)[:, None]
    t_i = np.arange(P, dtype=np.int64)[None, :]
    tri = (s_i <= t_i).astype(np.float32)
    cm = np.ascontiguousarray(
        np.repeat(tri[:, None, :], G, axis=1)).astype(bf)

    gidx = slot_map[:, :past].reshape(B, 8, P).transpose(2, 0, 1).reshape(P, B * 8)
    gidx = np.ascontiguousarray(gidx.astype(np.int32))

    rperm = np.zeros((P, P), np.float32)
    for d in range(D // 2):
        rperm[d + D // 2, d] = -1.0       # rot(q)[d] = -q[d+64] for d < 64
        rperm[d, d + D // 2] = 1.0        # rot(q)[d] = q[d-64] for d >= 64
    ident = np.eye(P, dtype=np.float32)
    ones = np.ones((P, P), np.float32)
    aux = np.ascontiguousarray(np.stack([rperm, ident, ones], axis=1)).astype(bf)

    def w_img(w_rows):          # [M, C] -> [P, 16, M] lhsT image
        return np.ascontiguousarray(
            w_rows.T.reshape(16, P, w_rows.shape[0]).transpose(1, 0, 2)
        ).astype(bf)

    in_maps = []
    for c in range(NCORES):
        wq_im = w_img(Wq[G * D * c:G * D * (c + 1), :])
        wk_im = w_img(Wkv[D * c:D * (c + 1), :])
        wv_im = w_img(Wkv[HKV * D + D * c:HKV * D + D * (c + 1), :])
        wo_im = np.ascontiguousarray(
            Wo[:, G * D * c:G * D * (c + 1)].T.reshape(G, P, C)
            .transpose(1, 0, 2)).astype(bf)
        in_maps.append({
            "xim": xim,
            "wq": wq_im, "wk": wk_im, "wv": wv_im, "wo": wo_im,
            "kp": np.ascontiguousarray(K_pool[:, c, :]).astype(bf),
            "vp": np.ascontiguousarray(V_pool[:, c, :]).astype(bf),
            "gidx": gidx,
            "rope": rope, "cmask": cm, "aux": aux,
        })
    return in_maps


_NC_CACHE = None


def kernel(**inputs):
    global _NC_CACHE
    in_maps = host_inputs(**inputs)
    if _NC_CACHE is None:
        _NC_CACHE = build_nc()
    res = run_bass_kernel_spmd(_NC_CACHE, in_maps, core_ids=list(range(NCORES)))
    y = res.results[0]["y"].astype(np.float32)
    for c in range(1, NCORES):
        y = y + res.results[c]["y"].astype(np.float32)
    return y.reshape(B, T, C)


# revision 4
# speedup vs baseline: 1.0226x; 1.0181x over previous
"""Trainium2 Bass kernel for paged causal self-attention (GQA + YaRN rope).

Fully bf16 dataflow (inputs cast + laid out as SBUF images on the host; all
matmuls bf16 with fp32 PSUM accumulation; tolerance is 2e-2, measured 8e-3):
- ACT/DVE ops span both q-heads per kv-head ([d, g, token] tiles); matmuls
  split per head (PSUM matmul outputs cap at 512 fp32 = one bank).
- Causally-masked diagonal chunks compute only the live token suffix per
  head, with one shared [128,128] lower-triangle multiplicative mask.
- exp-sums: full chunks are pair-summed on DVE, then ones-matmuls accumulate
  the denominator exactly in a [1,g,512] PSUM tile.
- Three-stage software pipeline keeps the in-order PE stream dense: scores
  run one chunk ahead of the tsum/av accumulation (hiding the ~1.1us exp
  latency); normalize of block i runs under attention of block i+1; out-proj
  of block i runs under block i+2. Activation tiles are per-block (Tile
  tracks dependencies per whole tile; monolithic tensors would serialize).
- reciprocal_approx_fast (18-bit) for the softmax denominator.
- DMA issue costs ~0.6us serial per dma_start on its HWDGE engine: few big
  DMAs, split across the sync AND scalar queues, critical-first (first
  matmul data = x quarter + wq half).

Sharding: tensor-parallel over heads. Core c (of 8) owns kv-head c and
q-heads 2c, 2c+1 for both batches; host sums the 8 bf16 partial y's in fp32.

The reference's scatter of new K/V into the pools is dead code w.r.t. the
returned output (slot_map is a permutation, so gathered past slots are
disjoint from the scattered new slots); new K/V are consumed directly from
SBUF and only the past 1024 slots per batch are gathered via indirect DMA,
unordered (softmax is permutation-invariant over fully-visible keys).
"""

import sys

sys.path.insert(0, "/opt/trn_rl_repo")

import ml_dtypes
import numpy as np

import concourse.bacc as bacc
import concourse.tile as tile
from concourse import mybir
from concourse.bass import IndirectOffsetOnAxis
from concourse.bass_utils import run_bass_kernel_spmd

BF = mybir.dt.bfloat16
F32 = mybir.dt.float32
I32 = mybir.dt.int32
EXP = mybir.ActivationFunctionType.Exp

B, T, PAST = 2, 1024, 1024
H, HKV, D = 16, 8, 128
G = H // HKV            # q heads per kv head
C = H * D               # 2048
TOTAL = PAST + T        # 2048
NB = B * T              # 2048 flattened tokens
NCORES = 8
P = 128
TB = 512                # token block
NTB = NB // TB          # 4


def _emit(tc, io):
    nc = tc.nc
    (xim, wq, wk, wv, wo, kp, vp, gidx, rope, cmask, aux, y) = io

    with (
        tc.tile_pool(name="const", bufs=1) as cp,
        tc.tile_pool(name="persist", bufs=1) as pp,
    ):
        # dma_start costs ~0.6us of serial issue time on its HWDGE engine, so
        # keep the DMA count low and split issues across sync AND scalar (both
        # are HWDGE on TRN2). Criticality order: kc=0..3 matmuls need
        # xq0/wq-half0 (sync) + wk/wv (scalar).
        wq2 = []
        for h in range(2):
            wqh = cp.tile([P, 8, G * P], BF, name=f"wqh{h}", tag=f"wqh{h}")
            wq2.append(wqh)
        wk_t = cp.tile([P, 16, P], BF)
        wv_t = cp.tile([P, 16, P], BF)
        nc.scalar.dma_start(wk_t[:], wk[:])
        nc.scalar.dma_start(wv_t[:], wv[:])
        nc.scalar.dma_start(wq2[1][:], wq[:, 8:16, :])
        nc.sync.dma_start(wq2[0][:], wq[:, 0:8, :])
        gidx_t = cp.tile([P, 2 * 8], I32)
        nc.scalar.dma_start(gidx_t[:], gidx[:])
        aux_t = cp.tile([P, 3, P], BF)          # rperm | ident | ones
        nc.scalar.dma_start(aux_t[:], aux[:])
        rope_t = cp.tile([P, 4, T], BF)         # cosq*s | sinq*s | cosk | sink
        nc.scalar.dma_start(rope_t[:], rope[:])

        # ---- persistent activations ----
        # per-token-block tiles: Tile tracks dependencies per whole tile, so
        # monolithic tensors would chain early attention blocks behind the
        # last block's projection/rope writes
        qT_blk = [pp.tile([P, G, TB], BF, name=f"qT{i}", tag=f"qT{i}")
                  for i in range(NTB)]      # roped q, [d, g, token]
        kT_blk = [pp.tile([P, TB], BF, name=f"kTn{i}", tag=f"kTn{i}")
                  for i in range(NTB)]      # new keys, [d, token]
        v_blk = [pp.tile([P, 4, P], BF, name=f"vn{i}", tag=f"vn{i}")
                 for i in range(NTB)]       # new values, [t%128, chunk, d]
        kT_past_b = [pp.tile([P, 8, P], BF, name=f"kTp{b}", tag=f"kTp{b}")
                     for b in range(B)]     # past keys, [d, chunk, s%128]
        # one att tile per 512-token block: whole-tile dependency tracking
        # would otherwise chain block i's out-proj behind block i+1's
        # normalize write
        att_blk = [pp.tile([P, G, TB], BF, name=f"att{i}", tag=f"att{i}")
                   for i in range(NTB)]
        kg = [None, None]
        vg = [None, None]
        for b in range(B):
            kg[b] = pp.tile([P, 8, P], BF, name=f"kg{b}", tag=f"kg{b}")
            vg[b] = pp.tile([P, 8, P], BF, name=f"vg{b}", tag=f"vg{b}")

        # ================= phase 1: projections + rope =================
        with (
            tc.tile_pool(name="xin", bufs=1) as xp,
            tc.tile_pool(name="rope_sb", bufs=2) as rp,
            tc.tile_pool(name="pproj", bufs=1, space="PSUM") as pjp,
            tc.tile_pool(name="prot", bufs=2, space="PSUM") as rpp,
            tc.tile_pool(name="ptr", bufs=2, space="PSUM") as trp,
        ):
            for tb in range(NTB):
                n0 = tb * TB
                b = tb // 2
                tpos = (tb % 2) * TB        # position-in-batch of block start

                if tb == 0:
                    # quarter-split so the first matmuls start ~3us in
                    xq = [None] * 4
                    for q4 in range(4):
                        xq[q4] = xp.tile([P, 4, TB], BF, name="xt",
                                         tag=f"xq{q4}")
                        nc.sync.dma_start(xq[q4][:],
                                          xim[tb, :, 4 * q4:4 * q4 + 4, :])
                    x_aps = [xq[kc // 4][:, kc % 4, :] for kc in range(16)]
                else:
                    xt = xp.tile([P, 16, TB], BF, name="xt16", tag="xt16",
                                 bufs=2)
                    nc.sync.dma_start(xt[:], xim[tb])
                    x_aps = [xt[:, kc, :] for kc in range(16)]

                q0p = pjp.tile([P, TB], F32, name="q0p", tag="q0")
                q1p = pjp.tile([P, TB], F32, name="q1p", tag="q1")
                kkp = pjp.tile([P, TB], F32, name="kkp", tag="kk")
                vvp = pjp.tile([P, TB], F32, name="vvp", tag="vv")
                for kc in range(16):
                    st = (kc == 0)
                    sp = (kc == 15)
                    x_ap = x_aps[kc]
                    nc.tensor.matmul(q0p[:], wq2[kc // 8][:, kc % 8, 0:P],
                                     x_ap, start=st, stop=sp)
                    nc.tensor.matmul(q1p[:], wq2[kc // 8][:, kc % 8, P:2 * P],
                                     x_ap, start=st, stop=sp)
                    nc.tensor.matmul(kkp[:], wk_t[:, kc, :], x_ap,
                                     start=st, stop=sp)
                    nc.tensor.matmul(vvp[:], wv_t[:, kc, :], x_ap,
                                     start=st, stop=sp)

                # rope for q0, q1, k
                for src, dslice, ci, si in (
                    (q0p, qT_blk[tb][:, 0, :], 0, 1),
                    (q1p, qT_blk[tb][:, 1, :], 0, 1),
                    (kkp, kT_blk[tb][:], 2, 3),
                ):
                    raw = rp.tile([P, TB], BF, name="raw", tag="raw")
                    nc.scalar.copy(raw[:], src[:])
                    rot = rpp.tile([P, TB], F32, name="rot", tag="rot")
                    nc.tensor.matmul(rot[:], aux_t[:, 0, :], raw[:],
                                     start=True, stop=True)
                    nc.vector.tensor_mul(dslice, raw[:],
                                         rope_t[:, ci, tpos:tpos + TB])
                    tmp = rp.tile([P, TB], BF, name="tmp", tag="tmp")
                    nc.vector.tensor_mul(tmp[:], rot[:],
                                         rope_t[:, si, tpos:tpos + TB])
                    nc.vector.tensor_add(dslice, dslice, tmp[:])

                # v: no rope; transpose [d, t] -> [t, d] in 128-chunks
                vraw = rp.tile([P, TB], BF, name="vraw", tag="vraw")
                nc.scalar.copy(vraw[:], vvp[:])
                for j4 in range(TB // P):
                    vt = trp.tile([P, P], BF, name="vt", tag="tr")
                    nc.tensor.transpose(vt[:], vraw[:, j4 * P:(j4 + 1) * P],
                                        aux_t[:, 1, :])
                    nc.vector.tensor_copy(v_blk[tb][:, j4, :], vt[:])

                if tb == 0:
                    # non-critical loads + past K/V gathers; emitted after
                    # tb0 so they don't contend with the startup-critical
                    # DMAs (gpsimd starts the gathers early regardless)
                    mask_t = cp.tile([P, G, P], BF)
                    nc.scalar.dma_start(mask_t[:], cmask[:])
                    wo_t = pp.tile([P, G, C], BF)
                    nc.scalar.dma_start(wo_t[:], wo[:])
                    for b2 in range(B):
                        for j in range(8):
                            # [P,1]-index gathers: the multi-column
                            # offset-AP form miscomputes on hardware
                            nc.gpsimd.indirect_dma_start(
                                out=kg[b2][:, j, :],
                                out_offset=None,
                                in_=kp[:, :],
                                in_offset=IndirectOffsetOnAxis(
                                    ap=gidx_t[:, 8 * b2 + j:8 * b2 + j + 1],
                                    axis=0),
                            )
                            nc.gpsimd.indirect_dma_start(
                                out=vg[b2][:, j, :],
                                out_offset=None,
                                in_=vp[:, :],
                                in_offset=IndirectOffsetOnAxis(
                                    ap=gidx_t[:, 8 * b2 + j:8 * b2 + j + 1],
                                    axis=0),
                            )

                if tb % 2 == 1:
                    # past K transpose [s, d] -> [d, s] for the batch whose
                    # projections just finished, so attention block (b, 0)
                    # is unblocked as early as possible
                    b2 = tb // 2
                    for j in range(8):
                        kt = trp.tile([P, P], BF, name="kt", tag="tr")
                        nc.tensor.transpose(kt[:], kg[b2][:, j, :],
                                            aux_t[:, 1, :])
                        if j % 2 == 0:
                            nc.vector.tensor_copy(kT_past_b[b2][:, j, :], kt[:])
                        else:
                            nc.scalar.copy(kT_past_b[b2][:, j, :], kt[:])

        # ================= phase 2: attention + output proj =================
        with (
            tc.tile_pool(name="exps", bufs=4) as ep,
            tc.tile_pool(name="avsb", bufs=2) as ab,
            tc.tile_pool(name="sums", bufs=2) as sp_,
            tc.tile_pool(name="ysb", bufs=3) as yp,
            tc.tile_pool(name="pbig", bufs=2, space="PSUM") as scp,
            tc.tile_pool(name="pav", bufs=1, space="PSUM") as avp,
            tc.tile_pool(name="psum1", bufs=1, space="PSUM") as s1p,
        ):
            def emit_attention(b, tbq):
                q_ap = qT_blk[2 * b + tbq]      # [d, g, tok]
                njnew = 4 * tbq + 4
                nch = 8 + njnew
                nfull = 8 + (4 if tbq == 1 else 0)   # chunks safe to pair-sum

                # (k_ap, v_ap, tok_off or None)
                chunks = [(kT_past_b[b][:, j, :], vg[b][:, j, :], None)
                          for j in range(8)]
                for j in range(njnew):
                    tbx = 2 * b + j // 4
                    ri = j - 4 * tbq
                    chunks.append((kT_blk[tbx][:, (j % 4) * P:(j % 4 + 1) * P],
                                   v_blk[tbx][:, j % 4, :],
                                   ri * P if ri >= 0 else None))

                tsum = s1p.tile([1, G, TB], F32, name="tsum", tag="t1")
                av = avp.tile([P, G, TB], F32, name="av", tag="av")

                # matmul PSUM outputs are capped at 512 fp32 (one bank), so
                # matmuls split per head; ACT/DVE ops span both heads.
                # Scores run one chunk ahead of tsum/av so the in-order PE
                # stream never waits on the ~1.1us exp latency.
                def emit_scores(k_ap, off):
                    if off is None or off == 0:
                        s_ps = scp.tile([P, G, TB], F32, name="s_ps", tag="s")
                        for g in range(G):
                            nc.tensor.matmul(s_ps[:, g, :], k_ap, q_ap[:, g, :],
                                             start=True, stop=True)
                        e = ep.tile([P, G, TB], BF, name="e", tag="e")
                        nc.scalar.activation(e[:], s_ps[:], EXP)
                        if off == 0:
                            nc.vector.tensor_mul(e[:, :, 0:P], e[:, :, 0:P],
                                                 mask_t[:])
                    else:
                        L = TB - off
                        s_ps = scp.tile([P, G, TB], F32, name="s_ps", tag="s")
                        e = ep.tile([P, G, TB], BF, name="e", tag="e")
                        for g in range(G):
                            nc.tensor.matmul(s_ps[:, g, 0:L],
                                             k_ap, q_ap[:, g, off:TB],
                                             start=True, stop=True)
                            nc.scalar.activation(e[:, g, 0:L],
                                                 s_ps[:, g, 0:L], EXP)
                        nc.vector.tensor_mul(e[:, :, 0:P], e[:, :, 0:P],
                                             mask_t[:])
                    return e

                def emit_av(e, v_ap, off, st, sp):
                    if off is None or off == 0:
                        for g in range(G):
                            nc.tensor.matmul(av[:, g, :], v_ap, e[:, g, :],
                                             start=st, stop=sp,
                                             skip_group_check=True)
                    else:
                        L = TB - off
                        for g in range(G):
                            nc.tensor.matmul(av[:, g, off:TB],
                                             v_ap, e[:, g, 0:L],
                                             start=st, stop=sp,
                                             skip_group_check=True)

                def emit_tsum(e_ap, off, st, sp):
                    if off is None or off == 0:
                        for g in range(G):
                            nc.tensor.matmul(tsum[:, g, :], aux_t[:, 2, 0:1],
                                             e_ap[:, g, :], start=st, stop=sp,
                                             skip_group_check=True)
                    else:
                        L = TB - off
                        for g in range(G):
                            nc.tensor.matmul(tsum[:, g, off:TB],
                                             aux_t[:, 2, 0:1], e_ap[:, g, 0:L],
                                             start=st, stop=sp,
                                             skip_group_check=True)

                # full-chunk pairs are pre-summed on DVE so the ones-reduction
                # runs half as many PE matmuls; all deferred emissions keep
                # the PE a chunk ahead of the exp/pair-add latency
                ntsum = nfull // 2 + (nch - nfull)
                tsum_done = 0
                tsum_q = []     # (e_ap, off)
                pend_av = None  # (e, v_ap, off, st)
                prev_e = None
                for ci, (k_ap, v_ap, off) in enumerate(chunks):
                    e = emit_scores(k_ap, off)
                    if tsum_q:
                        ea, eoff = tsum_q.pop(0)
                        emit_tsum(ea, eoff, tsum_done == 0, False)
                        tsum_done += 1
                    if pend_av is not None:
                        emit_av(pend_av[0], pend_av[1], pend_av[2],
                                pend_av[3], False)
                    pend_av = (e, v_ap, off, ci == 0)
                    if ci < nfull:
                        if ci % 2 == 1:
                            epair = ab.tile([P, G, TB], BF, name="epair",
                                            tag="epair", bufs=2)
                            nc.vector.tensor_add(epair[:], prev_e[:], e[:])
                            tsum_q.append((epair, None))
                        prev_e = e
                    else:
                        tsum_q.append((e, off))
                emit_av(pend_av[0], pend_av[1], pend_av[2], pend_av[3], True)
                for ea, eoff in tsum_q:
                    tsum_done += 1
                    emit_tsum(ea, eoff, False, tsum_done == ntsum)

                # denominator reciprocal + av evacuation (frees the PSUM
                # slots); the broadcast+normalize runs one block later so
                # the in-order PE stream never waits on this chain
                rinv32 = sp_.tile([1, G, TB], F32, name="rinv32", tag="ri32")
                nc.vector.reciprocal_approx_fast(rinv32[:], tsum[:])
                rinvb = sp_.tile([1, G, TB], BF, name="rinvb", tag="rib")
                nc.vector.tensor_copy(rinvb[:], rinv32[:])
                av_sb = ab.tile([P, G, TB], BF, name="av_sb", tag="avsb")
                nc.scalar.copy(av_sb[:], av[:])
                return rinvb, av_sb

            def emit_norm(blki, rinvb, av_sb):
                rbc = scp.tile([P, G, TB], F32, name="rbc", tag="s")
                for g in range(G):
                    nc.tensor.matmul(rbc[:, g, :], aux_t[0:1, 2, :],
                                     rinvb[:, g, :], start=True, stop=True)
                rbs = sp_.tile([P, G, TB], BF, name="rbs", tag="rbs")
                nc.scalar.copy(rbs[:], rbc[:])
                nc.vector.tensor_mul(att_blk[blki][:], av_sb[:], rbs[:])

            def emit_outproj(blki):
                b, tbq = divmod(blki, 2)
                t0 = b * T + tbq * TB
                att_t = att_blk[blki]
                for tc4 in range(4):
                    tt0 = t0 + tc4 * P
                    ysb = yp.tile([P, C], BF, name="ysbt", tag="ysbt")
                    for half in range(2):
                        yps = scp.tile([P, 2 * TB], F32, name="yps", tag="s")
                        for cb2 in range(2):
                            o0 = (2 * half + cb2) * TB
                            for g in range(G):
                                nc.tensor.matmul(yps[:, cb2 * TB:(cb2 + 1) * TB],
                                                 att_t[:, g, tc4 * P:tc4 * P + P],
                                                 wo_t[:, g, o0:o0 + TB],
                                                 start=(g == 0), stop=(g == 1))
                        dst = ysb[:, half * 2 * TB:(half + 1) * 2 * TB]
                        nc.vector.tensor_copy(dst, yps[:])
                    nc.sync.dma_start(y[tt0:tt0 + P, :], ysb[:])

            # two-deep software pipeline: normalize of block i runs under
            # attention of block i+1, out-proj of block i under block i+2,
            # so the PE stream never waits on the denominator chain
            blocks = [(b, tbq) for b in range(B) for tbq in range(2)]
            saved = []
            for i, blk in enumerate(blocks):
                saved.append(emit_attention(*blk))
                if i >= 1:
                    emit_norm(i - 1, *saved[i - 1])
                if i >= 2:
                    emit_outproj(i - 2)
            emit_norm(3, *saved[3])
            emit_outproj(2)
            emit_outproj(3)


def build_nc():
    nc = bacc.Bacc("TRN2")
    xim = nc.dram_tensor("xim", [NTB, P, 16, TB], BF, kind="ExternalInput")
    wq = nc.dram_tensor("wq", [P, 16, G * P], BF, kind="ExternalInput")
    wk = nc.dram_tensor("wk", [P, 16, P], BF, kind="ExternalInput")
    wv = nc.dram_tensor("wv", [P, 16, P], BF, kind="ExternalInput")
    wo = nc.dram_tensor("wo", [P, G, C], BF, kind="ExternalInput")
    kp = nc.dram_tensor("kp", [B * TOTAL, D], BF, kind="ExternalInput")
    vp = nc.dram_tensor("vp", [B * TOTAL, D], BF, kind="ExternalInput")
    gidx = nc.dram_tensor("gidx", [P, B * 8], I32, kind="ExternalInput")
    rope = nc.dram_tensor("rope", [P, 4, T], BF, kind="ExternalInput")
    cmask = nc.dram_tensor("cmask", [P, G, P], BF, kind="ExternalInput")
    aux = nc.dram_tensor("aux", [P, 3, P], BF, kind="ExternalInput")
    y = nc.dram_tensor("y", [NB, C], BF, kind="ExternalOutput")
    io = (xim, wq, wk, wv, wo, kp, vp, gidx, rope, cmask, aux, y)
    with nc.allow_low_precision(reason="bf16 dataflow; tolerance is 2e-2"):
        with tile.TileContext(nc) as tc:
            _emit(tc, io)
    nc.compile()
    return nc


def host_inputs(x, Wq, Wkv, Wo, K_pool, V_pool, slot_map, past_len):
    bf = ml_dtypes.bfloat16
    x = np.asarray(x, dtype=np.float32)
    Wq = np.asarray(Wq, dtype=np.float32)
    Wkv = np.asarray(Wkv, dtype=np.float32)
    Wo = np.asarray(Wo, dtype=np.float32)
    K_pool = np.asarray(K_pool, dtype=np.float32)
    V_pool = np.asarray(V_pool, dtype=np.float32)
    slot_map = np.asarray(slot_map, dtype=np.int32)
    past = int(past_len)
    assert past == PAST, f"kernel hardcodes past_len={PAST}, got {past}"

    # x image: [tb, p, kc, t] with p the in-channel within 128-chunk kc
    xT = x.reshape(NB, C).T                                  # [C, NB]
    xim = np.ascontiguousarray(
        xT.reshape(16, P, NTB, TB).transpose(2, 1, 0, 3)).astype(bf)

    # rope tables; argument arithmetic mirrors the f32 ops of the reference
    idx = np.arange(D // 2, dtype=np.float32)
    inv = np.float32(1.0) / np.float32(10000.0) ** (idx / np.float32(D // 2))
    inv = inv.astype(np.float32)
    t = np.arange(past, past + T, dtype=np.float32)
    freqs = (t[:, None] * inv[None, :]).astype(np.float32)
    emb = np.concatenate([freqs, freqs], axis=1)
    cos = np.cos(emb).astype(np.float32)                     # [T, D]
    sin = np.sin(emb).astype(np.float32)
    qscale = np.float32(1.0) / np.sqrt(np.float32(D))
    rope = np.ascontiguousarray(np.stack(
        [cos.T * qscale, sin.T * qscale, cos.T, sin.T], axis=1)).astype(bf)

    # one lower-triangle-inclusive 0/1 block, duplicated per q-head
    s_i = np.arange(P, dtype=np.int64)[:, None]
    t_i = np.arange(P, dtype=np.int64)[None, :]
    tri = (s_i <= t_i).astype(np.float32)
    cm = np.ascontiguousarray(
        np.repeat(tri[:, None, :], G, axis=1)).astype(bf)

    gidx = slot_map[:, :past].reshape(B, 8, P).transpose(2, 0, 1).reshape(P, B * 8)
    gidx = np.ascontiguousarray(gidx.astype(np.int32))

    rperm = np.zeros((P, P), np.float32)
    for d in range(D // 2):
        rperm[d + D // 2, d] = -1.0       # rot(q)[d] = -q[d+64] for d < 64
        rperm[d, d + D // 2] = 1.0        # rot(q)[d] = q[d-64] for d >= 64
    ident = np.eye(P, dtype=np.float32)
    ones = np.ones((P, P), np.float32)
    aux = np.ascontiguousarray(np.stack([rperm, ident, ones], axis=1)).astype(bf)

    def w_img(w_rows):          # [M, C] -> [P, 16, M] lhsT image
        return np.ascontiguousarray(
            w_rows.T.reshape(16, P, w_rows.shape[0]).transpose(1, 0, 2)
        ).astype(bf)

    in_maps = []
    for c in range(NCORES):
        wq_im = w_img(Wq[G * D * c:G * D * (c + 1), :])
        wk_im = w_img(Wkv[D * c:D * (c + 1), :])
        wv_im = w_img(Wkv[HKV * D + D * c:HKV * D + D * (c + 1), :])
        wo_im = np.ascontiguousarray(
            Wo[:, G * D * c:G * D * (c + 1)].T.reshape(G, P, C)
            .transpose(1, 0, 2)).astype(bf)
        in_maps.append({
            "xim": xim,
            "wq": wq_im, "wk": wk_im, "wv": wv_im, "wo": wo_im,
            "kp": np.ascontiguousarray(K_pool[:, c, :]).astype(bf),
            "vp": np.ascontiguousarray(V_pool[:, c, :]).astype(bf),
            "gidx": gidx,
            "rope": rope, "cmask": cm, "aux": aux,
        })
    return in_maps


_NC_CACHE = None


def kernel(**inputs):
    global _NC_CACHE
    in_maps = host_inputs(**inputs)
    if _NC_CACHE is None:
        _NC_CACHE = build_nc()
    res = run_bass_kernel_spmd(_NC_CACHE, in_maps, core_ids=list(range(NCORES)))
    y = res.results[0]["y"].astype(np.float32)
    for c in range(1, NCORES):
        y = y + res.results[c]["y"].astype(np.float32)
    return y.reshape(B, T, C)


# revision 5
# speedup vs baseline: 1.0487x; 1.0255x over previous
"""Trainium2 Bass kernel for paged causal self-attention (GQA + YaRN rope).

v3 over v2:
- Both q-heads processed per matmul: qT/att hold [d, g, token]; scores, the
  exp-sum ones-reduction, and att@V run with 1024-wide moving operands.
- Causally-masked diagonal chunks compute only the live token suffix per
  head, with one shared [128,128] lower-triangle multiplicative mask.
- Output projection for block i is emitted after attention of block i+1, so
  the PE never stalls on the softmax-denominator chain.
- reciprocal_approx_fast (18-bit) replaces the 8-pass iterative reciprocal.
- av is evacuated to SBUF by ScalarE so its PSUM slot recycles quickly;
  score/broadcast/out-proj PSUM tiles share one rotating 2-bank pool.
- x/weight DMAs are split and emitted critical-first so the PE starts ~4us in.

Sharding: tensor-parallel over heads. Core c (of 8) owns kv-head c and
q-heads 2c, 2c+1 for both batches; host sums the 8 bf16 partial y's in fp32.

The reference's scatter of new K/V into the pools is dead code w.r.t. the
returned output (slot_map is a permutation, so gathered past slots are
disjoint from the scattered new slots); new K/V are consumed directly from
SBUF and only the past 1024 slots per batch are gathered via indirect DMA,
unordered (softmax is permutation-invariant over fully-visible keys).
"""

import sys

sys.path.insert(0, "/opt/trn_rl_repo")

import ml_dtypes
import numpy as np

import concourse.bacc as bacc
import concourse.tile as tile
from concourse import mybir
from concourse.bass import IndirectOffsetOnAxis
from concourse.bass_utils import run_bass_kernel_spmd

BF = mybir.dt.bfloat16
F32 = mybir.dt.float32
I32 = mybir.dt.int32
EXP = mybir.ActivationFunctionType.Exp

B, T, PAST = 2, 1024, 1024
H, HKV, D = 16, 8, 128
G = H // HKV            # q heads per kv head
C = H * D               # 2048
TOTAL = PAST + T        # 2048
NB = B * T              # 2048 flattened tokens
NCORES = 8
P = 128
TB = 512                # token block
NTB = NB // TB          # 4


def _emit(tc, io):
    nc = tc.nc
    (xim, wq, wk, wv, wo, kp, vp, gidx, rope, cmask, aux, y) = io

    with (
        tc.tile_pool(name="const", bufs=1) as cp,
        tc.tile_pool(name="persist", bufs=1) as pp,
    ):
        # dma_start costs ~0.6us of serial issue time on its HWDGE engine, so
        # keep the DMA count low and split issues across sync AND scalar (both
        # are HWDGE on TRN2). Criticality order: kc=0..3 matmuls need
        # xq0/wq-half0 (sync) + wk/wv (scalar).
        wq2 = []
        for h in range(2):
            wqh = cp.tile([P, 8, G * P], BF, name=f"wqh{h}", tag=f"wqh{h}")
            wq2.append(wqh)
        wk_t = cp.tile([P, 16, P], BF)
        wv_t = cp.tile([P, 16, P], BF)
        nc.scalar.dma_start(wk_t[:], wk[:])
        nc.scalar.dma_start(wv_t[:], wv[:])
        nc.scalar.dma_start(wq2[1][:], wq[:, 8:16, :])
        nc.sync.dma_start(wq2[0][:], wq[:, 0:8, :])
        gidx_t = cp.tile([P, 2 * 8], I32)
        nc.scalar.dma_start(gidx_t[:], gidx[:])
        aux_t = cp.tile([P, 3, P], BF)          # rperm | ident | ones
        nc.scalar.dma_start(aux_t[:], aux[:])
        rope_t = cp.tile([P, 4, T], BF)         # cosq*s | sinq*s | cosk | sink
        nc.scalar.dma_start(rope_t[:], rope[:])

        # ---- persistent activations ----
        # per-token-block tiles: Tile tracks dependencies per whole tile, so
        # monolithic tensors would chain early attention blocks behind the
        # last block's projection/rope writes
        qT_blk = [pp.tile([P, G, TB], BF, name=f"qT{i}", tag=f"qT{i}")
                  for i in range(NTB)]      # roped q, [d, g, token]
        kT_blk = [pp.tile([P, TB], BF, name=f"kTn{i}", tag=f"kTn{i}")
                  for i in range(NTB)]      # new keys, [d, token]
        v_blk = [pp.tile([P, 4, P], BF, name=f"vn{i}", tag=f"vn{i}")
                 for i in range(NTB)]       # new values, [t%128, chunk, d]
        kT_past_b = [pp.tile([P, 8, P], BF, name=f"kTp{b}", tag=f"kTp{b}")
                     for b in range(B)]     # past keys, [d, chunk, s%128]
        # one att tile per 512-token block: whole-tile dependency tracking
        # would otherwise chain block i's out-proj behind block i+1's
        # normalize write
        att_blk = [pp.tile([P, G, TB], BF, name=f"att{i}", tag=f"att{i}")
                   for i in range(NTB)]
        kg = [None, None]
        vg = [None, None]
        for b in range(B):
            kg[b] = pp.tile([P, 8, P], BF, name=f"kg{b}", tag=f"kg{b}")
            vg[b] = pp.tile([P, 8, P], BF, name=f"vg{b}", tag=f"vg{b}")

        # ================= phase 1: projections + rope =================
        with (
            tc.tile_pool(name="xin", bufs=1) as xp,
            tc.tile_pool(name="rope_sb", bufs=2) as rp,
            tc.tile_pool(name="pproj", bufs=1, space="PSUM") as pjp,
            tc.tile_pool(name="prot", bufs=2, space="PSUM") as rpp,
            tc.tile_pool(name="ptr", bufs=2, space="PSUM") as trp,
        ):
            for tb in range(NTB):
                n0 = tb * TB
                b = tb // 2
                tpos = (tb % 2) * TB        # position-in-batch of block start

                if tb == 0:
                    # quarter-split so the first matmuls start ~3us in
                    xq = [None] * 4
                    for q4 in range(4):
                        xq[q4] = xp.tile([P, 4, TB], BF, name="xt",
                                         tag=f"xq{q4}")
                        nc.sync.dma_start(xq[q4][:],
                                          xim[tb, :, 4 * q4:4 * q4 + 4, :])
                    x_aps = [xq[kc // 4][:, kc % 4, :] for kc in range(16)]
                else:
                    xt = xp.tile([P, 16, TB], BF, name="xt16", tag="xt16",
                                 bufs=2)
                    nc.sync.dma_start(xt[:], xim[tb])
                    x_aps = [xt[:, kc, :] for kc in range(16)]

                q0p = pjp.tile([P, TB], F32, name="q0p", tag="q0")
                q1p = pjp.tile([P, TB], F32, name="q1p", tag="q1")
                kkp = pjp.tile([P, TB], F32, name="kkp", tag="kk")
                vvp = pjp.tile([P, TB], F32, name="vvp", tag="vv")
                for kc in range(16):
                    st = (kc == 0)
                    sp = (kc == 15)
                    x_ap = x_aps[kc]
                    nc.tensor.matmul(q0p[:], wq2[kc // 8][:, kc % 8, 0:P],
                                     x_ap, start=st, stop=sp)
                    nc.tensor.matmul(q1p[:], wq2[kc // 8][:, kc % 8, P:2 * P],
                                     x_ap, start=st, stop=sp)
                    nc.tensor.matmul(kkp[:], wk_t[:, kc, :], x_ap,
                                     start=st, stop=sp)
                    nc.tensor.matmul(vvp[:], wv_t[:, kc, :], x_ap,
                                     start=st, stop=sp)

                # rope for q0, q1, k
                for src, dslice, ci, si in (
                    (q0p, qT_blk[tb][:, 0, :], 0, 1),
                    (q1p, qT_blk[tb][:, 1, :], 0, 1),
                    (kkp, kT_blk[tb][:], 2, 3),
                ):
                    raw = rp.tile([P, TB], BF, name="raw", tag="raw")
                    nc.scalar.copy(raw[:], src[:])
                    rot = rpp.tile([P, TB], F32, name="rot", tag="rot")
                    nc.tensor.matmul(rot[:], aux_t[:, 0, :], raw[:],
                                     start=True, stop=True)
                    nc.vector.tensor_mul(dslice, raw[:],
                                         rope_t[:, ci, tpos:tpos + TB])
                    tmp = rp.tile([P, TB], BF, name="tmp", tag="tmp")
                    nc.vector.tensor_mul(tmp[:], rot[:],
                                         rope_t[:, si, tpos:tpos + TB])
                    nc.vector.tensor_add(dslice, dslice, tmp[:])

                # v: no rope; transpose [d, t] -> [t, d] in 128-chunks
                vraw = rp.tile([P, TB], BF, name="vraw", tag="vraw")
                nc.scalar.copy(vraw[:], vvp[:])
                for j4 in range(TB // P):
                    vt = trp.tile([P, P], BF, name="vt", tag="tr")
                    nc.tensor.transpose(vt[:], vraw[:, j4 * P:(j4 + 1) * P],
                                        aux_t[:, 1, :])
                    nc.vector.tensor_copy(v_blk[tb][:, j4, :], vt[:])

                if tb == 0:
                    # non-critical loads + past K/V gathers; emitted after
                    # tb0 so they don't contend with the startup-critical
                    # DMAs (gpsimd starts the gathers early regardless)
                    mask_t = cp.tile([P, G, P], BF)
                    nc.scalar.dma_start(mask_t[:], cmask[:])
                    wo_t = pp.tile([P, G, C], BF)
                    nc.scalar.dma_start(wo_t[:], wo[:])
                    for b2 in range(B):
                        for j in range(8):
                            # [P,1]-index gathers: the multi-column
                            # offset-AP form miscomputes on hardware
                            nc.gpsimd.indirect_dma_start(
                                out=kg[b2][:, j, :],
                                out_offset=None,
                                in_=kp[:, :],
                                in_offset=IndirectOffsetOnAxis(
                                    ap=gidx_t[:, 8 * b2 + j:8 * b2 + j + 1],
                                    axis=0),
                            )
                            nc.gpsimd.indirect_dma_start(
                                out=vg[b2][:, j, :],
                                out_offset=None,
                                in_=vp[:, :],
                                in_offset=IndirectOffsetOnAxis(
                                    ap=gidx_t[:, 8 * b2 + j:8 * b2 + j + 1],
                                    axis=0),
                            )

                if tb % 2 == 1:
                    # past K transpose [s, d] -> [d, s] for the batch whose
                    # projections just finished, so attention block (b, 0)
                    # is unblocked as early as possible
                    b2 = tb // 2
                    for j in range(8):
                        kt = trp.tile([P, P], BF, name="kt", tag="tr")
                        nc.tensor.transpose(kt[:], kg[b2][:, j, :],
                                            aux_t[:, 1, :])
                        if j % 2 == 0:
                            nc.vector.tensor_copy(kT_past_b[b2][:, j, :], kt[:])
                        else:
                            nc.scalar.copy(kT_past_b[b2][:, j, :], kt[:])

        # ================= phase 2: attention + output proj =================
        with (
            tc.tile_pool(name="exps", bufs=4) as ep,
            tc.tile_pool(name="avsb", bufs=2) as ab,
            tc.tile_pool(name="sums", bufs=2) as sp_,
            tc.tile_pool(name="ysb", bufs=3) as yp,
            tc.tile_pool(name="pbig", bufs=2, space="PSUM") as scp,
            tc.tile_pool(name="pav", bufs=1, space="PSUM") as avp,
            tc.tile_pool(name="psum1", bufs=1, space="PSUM") as s1p,
        ):
            def emit_attention(b, tbq):
                q_ap = qT_blk[2 * b + tbq]      # [d, g, tok]
                njnew = 4 * tbq + 4
                nch = 8 + njnew
                nfull = 8 + (4 if tbq == 1 else 0)   # chunks safe to pair-sum

                # (k_ap, v_ap, tok_off or None)
                chunks = [(kT_past_b[b][:, j, :], vg[b][:, j, :], None)
                          for j in range(8)]
                for j in range(njnew):
                    tbx = 2 * b + j // 4
                    ri = j - 4 * tbq
                    chunks.append((kT_blk[tbx][:, (j % 4) * P:(j % 4 + 1) * P],
                                   v_blk[tbx][:, j % 4, :],
                                   ri * P if ri >= 0 else None))

                tsum = s1p.tile([1, G, TB], F32, name="tsum", tag="t1")
                av = avp.tile([P, G, TB], F32, name="av", tag="av")

                # matmul PSUM outputs are capped at 512 fp32 (one bank), so
                # matmuls split per head; ACT/DVE ops span both heads.
                # Scores run one chunk ahead of tsum/av so the in-order PE
                # stream never waits on the ~1.1us exp latency.
                def emit_scores(k_ap, off):
                    if off is None or off == 0:
                        s_ps = scp.tile([P, G, TB], F32, name="s_ps", tag="s")
                        for g in range(G):
                            nc.tensor.matmul(s_ps[:, g, :], k_ap, q_ap[:, g, :],
                                             start=True, stop=True)
                        e = ep.tile([P, G, TB], BF, name="e", tag="e")
                        nc.scalar.activation(e[:], s_ps[:], EXP)
                        if off == 0:
                            nc.vector.tensor_mul(e[:, :, 0:P], e[:, :, 0:P],
                                                 mask_t[:])
                    else:
                        L = TB - off
                        s_ps = scp.tile([P, G, TB], F32, name="s_ps", tag="s")
                        e = ep.tile([P, G, TB], BF, name="e", tag="e")
                        for g in range(G):
                            nc.tensor.matmul(s_ps[:, g, 0:L],
                                             k_ap, q_ap[:, g, off:TB],
                                             start=True, stop=True)
                            nc.scalar.activation(e[:, g, 0:L],
                                                 s_ps[:, g, 0:L], EXP)
                        nc.vector.tensor_mul(e[:, :, 0:P], e[:, :, 0:P],
                                             mask_t[:])
                    return e

                def emit_av(e, v_ap, off, st, sp):
                    if off is None or off == 0:
                        for g in range(G):
                            nc.tensor.matmul(av[:, g, :], v_ap, e[:, g, :],
                                             start=st, stop=sp,
                                             skip_group_check=True)
                    else:
                        L = TB - off
                        for g in range(G):
                            nc.tensor.matmul(av[:, g, off:TB],
                                             v_ap, e[:, g, 0:L],
                                             start=st, stop=sp,
                                             skip_group_check=True)

                def emit_tsum(e_ap, off, st, sp):
                    if off is None or off == 0:
                        for g in range(G):
                            nc.tensor.matmul(tsum[:, g, :], aux_t[:, 2, 0:1],
                                             e_ap[:, g, :], start=st, stop=sp,
                                             skip_group_check=True)
                    else:
                        L = TB - off
                        for g in range(G):
                            nc.tensor.matmul(tsum[:, g, off:TB],
                                             aux_t[:, 2, 0:1], e_ap[:, g, 0:L],
                                             start=st, stop=sp,
                                             skip_group_check=True)

                # full-chunk pairs are pre-summed on DVE so the ones-reduction
                # runs half as many PE matmuls; all deferred emissions keep
                # the PE a chunk ahead of the exp/pair-add latency
                ntsum = nfull // 2 + (nch - nfull)
                tsum_done = 0
                tsum_q = []     # (e_ap, off)
                pend_av = None  # (e, v_ap, off, st)
                prev_e = None
                for ci, (k_ap, v_ap, off) in enumerate(chunks):
                    e = emit_scores(k_ap, off)
                    if tsum_q:
                        ea, eoff = tsum_q.pop(0)
                        emit_tsum(ea, eoff, tsum_done == 0, False)
                        tsum_done += 1
                    if pend_av is not None:
                        emit_av(pend_av[0], pend_av[1], pend_av[2],
                                pend_av[3], False)
                    pend_av = (e, v_ap, off, ci == 0)
                    if ci < nfull:
                        if ci % 2 == 1:
                            epair = ab.tile([P, G, TB], BF, name="epair",
                                            tag="epair", bufs=2)
                            nc.vector.tensor_add(epair[:], prev_e[:], e[:])
                            tsum_q.append((epair, None))
                        prev_e = e
                    else:
                        tsum_q.append((e, off))
                emit_av(pend_av[0], pend_av[1], pend_av[2], pend_av[3], True)
                for ea, eoff in tsum_q:
                    tsum_done += 1
                    emit_tsum(ea, eoff, False, tsum_done == ntsum)

                # denominator reciprocal + av evacuation (frees the PSUM
                # slots); the broadcast+normalize runs one block later so
                # the in-order PE stream never waits on this chain
                rinv32 = sp_.tile([1, G, TB], F32, name="rinv32", tag="ri32")
                nc.vector.reciprocal_approx_fast(rinv32[:], tsum[:])
                rinvb = sp_.tile([1, G, TB], BF, name="rinvb", tag="rib")
                nc.vector.tensor_copy(rinvb[:], rinv32[:])
                av_sb = ab.tile([P, G, TB], BF, name="av_sb", tag="avsb")
                nc.scalar.copy(av_sb[:], av[:])
                return rinvb, av_sb

            def emit_norm(blki, rinvb, av_sb):
                rbc = scp.tile([P, G, TB], F32, name="rbc", tag="s")
                for g in range(G):
                    nc.tensor.matmul(rbc[:, g, :], aux_t[0:1, 2, :],
                                     rinvb[:, g, :], start=True, stop=True)
                rbs = sp_.tile([P, G, TB], BF, name="rbs", tag="rbs")
                nc.scalar.copy(rbs[:], rbc[:])
                nc.vector.tensor_mul(att_blk[blki][:], av_sb[:], rbs[:])

            def emit_outproj(blki):
                b, tbq = divmod(blki, 2)
                t0 = b * T + tbq * TB
                att_t = att_blk[blki]
                for tc4 in range(4):
                    tt0 = t0 + tc4 * P
                    ysb = yp.tile([P, C], BF, name="ysbt", tag="ysbt")
                    for half in range(2):
                        yps = scp.tile([P, 2 * TB], F32, name="yps", tag="s")
                        for cb2 in range(2):
                            o0 = (2 * half + cb2) * TB
                            for g in range(G):
                                nc.tensor.matmul(yps[:, cb2 * TB:(cb2 + 1) * TB],
                                                 att_t[:, g, tc4 * P:tc4 * P + P],
                                                 wo_t[:, g, o0:o0 + TB],
                                                 start=(g == 0), stop=(g == 1))
                        dst = ysb[:, half * 2 * TB:(half + 1) * 2 * TB]
                        if (tc4 + half) % 2 == 0:
                            nc.scalar.copy(dst, yps[:])
                        else:
                            nc.vector.tensor_copy(dst, yps[:])
                    nc.sync.dma_start(y[tt0:tt0 + P, :], ysb[:])

            # two-deep software pipeline: normalize of block i runs under
            # attention of block i+1, out-proj of block i under block i+2,
            # so the PE stream never waits on the denominator chain
            blocks = [(b, tbq) for b in range(B) for tbq in range(2)]
            saved = []
            for i, blk in enumerate(blocks):
                saved.append(emit_attention(*blk))
                if i >= 1:
                    emit_norm(i - 1, *saved[i - 1])
                if i >= 2:
                    emit_outproj(i - 2)
            emit_norm(3, *saved[3])
            emit_outproj(2)
            emit_outproj(3)


def build_nc():
    nc = bacc.Bacc("TRN2")
    xim = nc.dram_tensor("xim", [NTB, P, 16, TB], BF, kind="ExternalInput")
    wq = nc.dram_tensor("wq", [P, 16, G * P], BF, kind="ExternalInput")
    wk = nc.dram_tensor("wk", [P, 16, P], BF, kind="ExternalInput")
    wv = nc.dram_tensor("wv", [P, 16, P], BF, kind="ExternalInput")
    wo = nc.dram_tensor("wo", [P, G, C], BF, kind="ExternalInput")
    kp = nc.dram_tensor("kp", [B * TOTAL, D], BF, kind="ExternalInput")
    vp = nc.dram_tensor("vp", [B * TOTAL, D], BF, kind="ExternalInput")
    gidx = nc.dram_tensor("gidx", [P, B * 8], I32, kind="ExternalInput")
    rope = nc.dram_tensor("rope", [P, 4, T], BF, kind="ExternalInput")
    cmask = nc.dram_tensor("cmask", [P, G, P], BF, kind="ExternalInput")
    aux = nc.dram_tensor("aux", [P, 3, P], BF, kind="ExternalInput")
    y = nc.dram_tensor("y", [NB, C], BF, kind="ExternalOutput")
    io = (xim, wq, wk, wv, wo, kp, vp, gidx, rope, cmask, aux, y)
    with nc.allow_low_precision(reason="bf16 dataflow; tolerance is 2e-2"):
        with tile.TileContext(nc) as tc:
            _emit(tc, io)
    nc.compile()
    return nc


def host_inputs(x, Wq, Wkv, Wo, K_pool, V_pool, slot_map, past_len):
    bf = ml_dtypes.bfloat16
    x = np.asarray(x, dtype=np.float32)
    Wq = np.asarray(Wq, dtype=np.float32)
    Wkv = np.asarray(Wkv, dtype=np.float32)
    Wo = np.asarray(Wo, dtype=np.float32)
    K_pool = np.asarray(K_pool, dtype=np.float32)
    V_pool = np.asarray(V_pool, dtype=np.float32)
    slot_map = np.asarray(slot_map, dtype=np.int32)
    past = int(past_len)
    assert past == PAST, f"kernel hardcodes past_len={PAST}, got {past}"

    # x image: [tb, p, kc, t] with p the in-channel within 128-chunk kc
    xT = x.reshape(NB, C).T                                  # [C, NB]
    xim = np.ascontiguousarray(
        xT.reshape(16, P, NTB, TB).transpose(2, 1, 0, 3)).astype(bf)

    # rope tables; argument arithmetic mirrors the f32 ops of the reference
    idx = np.arange(D // 2, dtype=np.float32)
    inv = np.float32(1.0) / np.float32(10000.0) ** (idx / np.float32(D // 2))
    inv = inv.astype(np.float32)
    t = np.arange(past, past + T, dtype=np.float32)
    freqs = (t[:, None] * inv[None, :]).astype(np.float32)
    emb = np.concatenate([freqs, freqs], axis=1)
    cos = np.cos(emb).astype(np.float32)                     # [T, D]
    sin = np.sin(emb).astype(np.float32)
    qscale = np.float32(1.0) / np.sqrt(np.float32(D))
    rope = np.ascontiguousarray(np.stack(
        [cos.T * qscale, sin.T * qscale, cos.T, sin.T], axis=1)).astype(bf)

    # one lower-triangle-inclusive 0/1 block, duplicated per q-head
    s_i = np.arange(P, dtype=np.int64)[:, None]
    t_i = np.arange(P, dtype=np.int64)[None, :]
    tri = (s_i <= t_i).astype(np.float32)
    cm = np.ascontiguousarray(
        np.repeat(tri[:, None, :], G, axis=1)).astype(bf)

    gidx = slot_map[:, :past].reshape(B, 8, P).transpose(2, 0, 1).reshape(P, B * 8)
    gidx = np.ascontiguousarray(gidx.astype(np.int32))

    rperm = np.zeros((P, P), np.float32)
    for d in range(D // 2):
        rperm[d + D // 2, d] = -1.0       # rot(q)[d] = -q[d+64] for d < 64
        rperm[d, d + D // 2] = 1.0        # rot(q)[d] = q[d-64] for d >= 64
    ident = np.eye(P, dtype=np.float32)
    ones = np.ones((P, P), np.float32)
    aux = np.ascontiguousarray(np.stack([rperm, ident, ones], axis=1)).astype(bf)

    def w_img(w_rows):          # [M, C] -> [P, 16, M] lhsT image
        return np.ascontiguousarray(
            w_rows.T.reshape(16, P, w_rows.shape[0]).transpose(1, 0, 2)
        ).astype(bf)

    in_maps = []
    for c in range(NCORES):
        wq_im = w_img(Wq[G * D * c:G * D * (c + 1), :])
        wk_im = w_img(Wkv[D * c:D * (c + 1), :])
        wv_im = w_img(Wkv[HKV * D + D * c:HKV * D + D * (c + 1), :])
        wo_im = np.ascontiguousarray(
            Wo[:, G * D * c:G * D * (c + 1)].T.reshape(G, P, C)
            .transpose(1, 0, 2)).astype(bf)
        in_maps.append({
            "xim": xim,
            "wq": wq_im, "wk": wk_im, "wv": wv_im, "wo": wo_im,
            "kp": np.ascontiguousarray(K_pool[:, c, :]).astype(bf),
            "vp": np.ascontiguousarray(V_pool[:, c, :]).astype(bf),
            "gidx": gidx,
            "rope": rope, "cmask": cm, "aux": aux,
        })
    return in_maps


_NC_CACHE = None


def kernel(**inputs):
    global _NC_CACHE
    in_maps = host_inputs(**inputs)
    if _NC_CACHE is None:
        _NC_CACHE = build_nc()
    res = run_bass_kernel_spmd(_NC_CACHE, in_maps, core_ids=list(range(NCORES)))
    y = res.results[0]["y"].astype(np.float32)
    for c in range(1, NCORES):
        y = y + res.results[c]["y"].astype(np.float32)
    return y.reshape(B, T, C)
